# revision 18
# baseline (speedup 1.0000x reference)
"""Trainium2 Bass kernel for nn_BinReLUConvBN (dense_cnn).

Full module: out = prelu(BN2(conv_g16(sign(BN1(x)), sign(w)*sw)) + x)

Sharding: channels C=512 -> 8 cores x 64 ch (= 2 conv groups of 32).
BN stats are per-channel and grouped conv respects channel blocks, so
there is NO cross-core communication at all.

Per-core layout (N=64, C=64): n = 2*r + L2, r in [0,32), L2 in [0,2).
Partition p = 64*L2 + 32*g + ci  -- both batch lanes x both groups x 32
channels on the 128 partitions.

Key property: the grouped conv maps channel block g -> channel block g,
so with this layout the matmul OUTPUT partition (L2, g, co) coincides
with the input/residual partition (L2, g, ci). No transposes anywhere.

Conv: one 128x128 block-diagonal fp8 weight matrix (sign(w) placed at
diagonal blocks (L2,g)==(L2',g'), zeros elsewhere) contracts ALL 128
partitions per instruction. The 9 taps are issued as 4 fp8 DoubleRow
matmuls (2 taps each, k-tile dim = tap pair with constant stride in the
padded 34x34 h image) + 1 plain matmul, accumulating in PSUM.
PE streaming cost per round: 2 chunks * (4*256 + 512) = 3072 cycles --
12x less sequencer time than the 16x(32x32 subtile) formulation.

Drain: psum -> yt (fp16, partition-aligned) split between DVE and Act;
bn_stats for BN2 taken from yt. Final pass: ut = A*yt + x (DVE, into
PSUM), Prelu+bias on Act -> fp16 out, 32 big output DMAs.
HBM traffic/core: 16.8 MB in + 8.4 MB out.
"""

import sys

for _p in ("/opt/trn_rl_repo",):
    if _p not in sys.path:
        sys.path.insert(0, _p)

import numpy as np

import concourse.bacc as bacc
import concourse.bass as bass
import concourse.mybir as mybir
import concourse.tile as tile

F32 = mybir.dt.float32
F16 = mybir.dt.float16
F8 = mybir.dt.float8e4
EPS = 1e-5

# per-core shapes
N, C = 64, 64  # batch, channels per core
G = 2          # conv groups per core (32 ch each)
R, NL = 32, 2  # rounds x lanes = N ; n = 2r + L2
HW = 1024
HP = 34 * 34   # padded h image size per (partition, slot)
HPS = HP + 4   # slot stride: +4 tail pad (tap windows of the last chunk overrun by 2)
# tap t=(di,dj), offset in padded image Delta = 34*di + dj
# DoubleRow pairs (t, t+1): (0,1) d=1, (2,3) d=32, (4,5) d=1, (6,7) d=1; tap 8 single
TAP_PAIRS = [(0, 1), (2, 32), (35, 1), (68, 1)]  # (Delta0, delta)
TAP8_OFF = 70  # (2,2)
# output row chunks per round: PSUM streams full 34-wide padded rows
# (contiguous), cols 32,33 of each row are garbage and skipped at drain
CHUNKS = [(0, 11), (11, 11), (22, 10)]  # (row0, nrows)


def build_nc(debug=False, loop_n=None, ablate=()):
    nc = bacc.Bacc(None, target_bir_lowering=False, debug=debug)

    x_d = nc.dram_tensor("x1", [128, R, HW], F32, kind="ExternalInput")
    w_d = nc.dram_tensor("w", [C, 32, 3, 3], F32, kind="ExternalInput")
    p_d = nc.dram_tensor("p", [5, C], F32, kind="ExternalInput")  # g1,b1,g2,b2,alpha
    out_d = nc.dram_tensor("out", [128, R, HW], F16, kind="ExternalOutput")

    with tile.TileContext(nc) as tc:
        _body(tc, nc, x_d, w_d, p_d, out_d, loop_n=loop_n, ablate=ablate)
    nc.compile()
    return nc


def _body(tc, nc, x_d, w_d, p_d, out_d, loop_n=None, ablate=()):
    pools = []

    def pool(**kw):
        p = tc.alloc_tile_pool(**kw)
        pools.append(p)
        return p

    big = pool(name="big", bufs=1)
    stgp = pool(name="stg", bufs=2)
    psp = [pool(name=f"ps{c}", bufs=2, space="PSUM") for c in range(3)]

    def emit():
        x = big.tile([128, R, 32, 32], F32, tag="x", name="x")
        yt = big.tile([128, R, HW], F16, tag="yt", name="yt")
        h = big.tile([128, 4, HPS], F8, tag="h", name="h")  # 4 slots of padded 34x34 + tail pad
        wblk = big.tile([128, 9, 128], F8, tag="wblk", name="wblk")  # block-diag lhsT per tap
        wsrc = big.tile([32, G, 9, 32], F32, tag="wsrc", name="wsrc")  # transposed w for sign
        ws = big.tile([64, 288], F32, tag="ws", name="ws")  # co-major w for sw stats (centered in-place)
        params = big.tile([64, 5], F32, tag="par", name="par")
        st = big.tile([128, 64, 6], F32, tag="st", name="st")  # shared BN1/BN2 bn_stats slots
        mv1 = big.tile([128, 2], F32, tag="mv1", name="mv1")
        mv2 = big.tile([128, 2], F32, tag="mv2", name="mv2")
        gth1 = big.tile([64, 2, NL], F32, tag="gth1", name="gth1")  # (stat, lane)
        gth2 = big.tile([64, 2, NL], F32, tag="gth2", name="gth2")
        sc = big.tile([64, 16], F32, tag="sc", name="sc")  # scalar scratch columns
        sb1 = big.tile([128, 2], F32, tag="sb1", name="sb1")  # scale,bias BN1 per partition
        abx = big.tile([128, 3], F32, tag="abx", name="abx")  # A,B,alpha per partition
        czero = big.tile([64, 2], F32, tag="czero", name="czero")  # col0: 0.0, col1: EPS
        t24 = big.tile([64, NL], F32, tag="t24", name="t24")
        aba = big.tile([64, 3], F32, tag="aba", name="aba")

        sync = nc.sync
        vec = nc.vector
        act = nc.scalar
        gp = nc.gpsimd
        DR = mybir.MatmulPerfMode.DoubleRow

        # ---------------- loads ----------------
        for k in range(8):
            sync.dma_start(
                out=x[:, 4 * k : 4 * k + 4, :, :],
                in_=bass.AP(x_d, 4 * k * HW, [[R * HW, 128], [HW, 4], [1, HW]]),
            )
        sync.dma_start(out=ws[:, :], in_=bass.AP(w_d, 0, [[288, 64], [1, 288]]))
        for g in range(G):
            sync.dma_start(
                out=wsrc[:, g, :, :],
                in_=bass.AP(w_d, g * 32 * 288, [[9, 32], [1, 9], [288, 32]]),
            )
        sync.dma_start(out=params[:, :], in_=bass.AP(p_d, 0, [[1, 64], [64, 5]]))

        vec.memset(czero[:, 0:1], 0.0)
        vec.memset(czero[:, 1:2], EPS)

        # ---------------- weights: block-diagonal sign lhsT; sw ----------------
        gp.memset(wblk[:, :, :], 0.0)
        for L in range(NL):
            for g in range(G):
                q = 64 * L + 32 * g
                act.sign(
                    wblk[q : q + 32, :, q : q + 32],
                    wsrc[:, g, :, :],
                    bias=czero[0:32, 0:1],
                )

        # sw: per-co center/unbiased-std/mean|.|
        vec.tensor_reduce(sc[:, 0:1], ws[:, :], mybir.AxisListType.X, mybir.AluOpType.add)
        vec.tensor_scalar_mul(sc[:, 0:1], sc[:, 0:1], 1.0 / 288.0)  # mean
        vec.tensor_scalar(
            ws[:, :], ws[:, :], sc[:, 0:1], None, mybir.AluOpType.subtract
        )  # centered (in-place)
        vec.tensor_reduce(
            sc[:, 1:2], ws[:, :], mybir.AxisListType.X, mybir.AluOpType.add,
            apply_absolute_value=True,
        )  # sum|d|
        vec.tensor_mul(ws[:, :], ws[:, :], ws[:, :])  # d^2 (in-place)
        vec.tensor_reduce(sc[:, 2:3], ws[:, :], mybir.AxisListType.X, mybir.AluOpType.add)
        # std = sqrt(ss/287); sw = (sum|d|/288) / std
        act.activation(sc[:, 3:4], sc[:, 2:3], mybir.ActivationFunctionType.Sqrt,
                       bias=czero[:, 0:1], scale=1.0 / 287.0)
        vec.reciprocal(sc[:, 4:5], sc[:, 3:4])
        vec.tensor_mul(sc[:, 5:6], sc[:, 1:2], sc[:, 4:5])
        vec.tensor_scalar_mul(sc[:, 5:6], sc[:, 5:6], 1.0 / 288.0)  # sw -> col5

        # ---------------- BN1 stats ----------------
        for r in range(R):
            for c in range(2):
                vec.bn_stats(
                    st[:, 2 * r + c, :],
                    x[:, r, 16 * c : 16 * c + 16, :].rearrange("p a b -> p (a b)"),
                )
        vec.bn_aggr(mv1[:, :], st[:, :, :])
        for L in range(NL):
            sync.dma_start(out=gth1[:, :, L : L + 1], in_=mv1[64 * L : 64 * L + 64, :])
        # per-channel mean/var from 2 equal-count lane partials
        vec.tensor_reduce(sc[:, 6:7], gth1[:, 0, :], mybir.AxisListType.X, mybir.AluOpType.add)
        vec.tensor_scalar_mul(sc[:, 6:7], sc[:, 6:7], 0.5)  # E[x] -> col6
        vec.tensor_mul(t24[:, :], gth1[:, 0, :], gth1[:, 0, :])
        vec.tensor_add(t24[:, :], t24[:, :], gth1[:, 1, :])
        vec.tensor_reduce(sc[:, 7:8], t24[:, :], mybir.AxisListType.X, mybir.AluOpType.add)
        vec.tensor_scalar_mul(sc[:, 7:8], sc[:, 7:8], 0.5)  # E[x^2]
        vec.tensor_mul(sc[:, 8:9], sc[:, 6:7], sc[:, 6:7])
        vec.tensor_sub(sc[:, 8:9], sc[:, 7:8], sc[:, 8:9])  # var -> col8
        act.activation(sc[:, 9:10], sc[:, 8:9], mybir.ActivationFunctionType.Sqrt,
                       bias=czero[:, 1:2], scale=1.0)
        vec.reciprocal(sc[:, 9:10], sc[:, 9:10])  # rsqrt(var+eps) -> col9
        vec.tensor_mul(sc[:, 10:11], sc[:, 9:10], params[:, 0:1])  # scale1 = g1*rv
        vec.tensor_mul(sc[:, 11:12], sc[:, 6:7], sc[:, 10:11])
        vec.tensor_sub(sc[:, 11:12], params[:, 1:2], sc[:, 11:12])  # bias1 = b1 - m*scale1
        for L in range(NL):
            sync.dma_start(out=sb1[64 * L : 64 * L + 64, 0:1], in_=sc[:, 10:11])
            sync.dma_start(out=sb1[64 * L : 64 * L + 64, 1:2], in_=sc[:, 11:12])

        # zero whole h once: borders + tail pad stay zero forever (sign only
        # ever rewrites the 32x32 interior)
        gp.memset(h[:, :, :], 0.0)

        ht = h[:, :, :].tensor

        def rhs_ap(s, row0, nrows, d0, dd, kt):
            # [128 part][kt pair @ dd][nrows*34 flat @ 1] -- full padded rows
            off = s * HPS + 34 * row0 + d0
            dims = [[HPS * 4, 128]]
            if kt:
                dims.append([dd, 2])
            dims.append([1, 34 * nrows])
            return bass.AP(ht, off, dims)

        # ---------------- sign + conv + psum drain, per round ----------------
        for r in range(R):
            s = r % 4
            act.activation(
                bass.AP(ht, s * HPS + 35, [[HPS * 4, 128], [34, 32], [1, 32]]),
                x[:, r, :, :],
                mybir.ActivationFunctionType.Sign,
                bias=sb1[:, 1:2],
                scale=sb1[:, 0:1],
            )
            pts = [
                psp[c].tile([128, nr, 34], F32, tag=f"pt{c}", name=f"pt{c}")
                for c, (r0, nr) in enumerate(CHUNKS)
            ]
            if "conv" not in ablate:
                for c, (r0, nr) in enumerate(CHUNKS):
                    po = pts[c][:, :, :].rearrange("p a b -> p (a b)")
                    for tb, (d0, dd) in enumerate(TAP_PAIRS):
                        nc.tensor.matmul(
                            po,
                            wblk[:, 2 * tb : 2 * tb + 2, :],
                            rhs_ap(s, r0, nr, d0, dd, True),
                            start=(tb == 0),
                            stop=False,
                            perf_mode=DR,
                        )
                    nc.tensor.matmul(
                        po,
                        wblk[:, 8, :],
                        rhs_ap(s, r0, nr, TAP8_OFF, 0, False),
                        start=False,
                        stop=True,
                    )
            # drain used cols (skip 32,33 of each row); c1 on Act for balance
            vec.tensor_copy(yt[:, r, 0 : 352], pts[0][:, :, 0:32])
            act.activation(yt[:, r, 352 : 704], pts[1][:, :, 0:32],
                           mybir.ActivationFunctionType.Copy, bias=0.0, scale=1.0)
            vec.tensor_copy(yt[:, r, 704 : 1024], pts[2][:, :, 0:32])
            vec.bn_stats(st[:, 2 * r, :], yt[:, r, 0:512])
            vec.bn_stats(st[:, 2 * r + 1, :], yt[:, r, 512:1024])

        # ---------------- BN2 -> A, B ----------------
        vec.bn_aggr(mv2[:, :], st[:, :, :])
        for L in range(NL):
            sync.dma_start(out=gth2[:, :, L : L + 1], in_=mv2[64 * L : 64 * L + 64, :])
        vec.tensor_reduce(sc[:, 12:13], gth2[:, 0, :], mybir.AxisListType.X, mybir.AluOpType.add)
        vec.tensor_scalar_mul(sc[:, 12:13], sc[:, 12:13], 0.5)  # E[y_raw]
        vec.tensor_mul(t24[:, :], gth2[:, 0, :], gth2[:, 0, :])
        vec.tensor_add(t24[:, :], t24[:, :], gth2[:, 1, :])
        vec.tensor_reduce(sc[:, 13:14], t24[:, :], mybir.AxisListType.X, mybir.AluOpType.add)
        vec.tensor_scalar_mul(sc[:, 13:14], sc[:, 13:14], 0.5)  # E[y^2]
        vec.tensor_mul(sc[:, 14:15], sc[:, 12:13], sc[:, 12:13])
        vec.tensor_sub(sc[:, 14:15], sc[:, 13:14], sc[:, 14:15])  # var_raw
        # rv2 = 1/sqrt(sw^2*var + eps); A = g2*sw*rv2; B = b2 - E*A
        vec.tensor_mul(sc[:, 15:16], sc[:, 5:6], sc[:, 5:6])
        vec.tensor_mul(sc[:, 15:16], sc[:, 15:16], sc[:, 14:15])
        act.activation(sc[:, 15:16], sc[:, 15:16], mybir.ActivationFunctionType.Sqrt,
                       bias=czero[:, 1:2], scale=1.0)
        vec.reciprocal(sc[:, 15:16], sc[:, 15:16])
        vec.tensor_mul(sc[:, 15:16], sc[:, 15:16], sc[:, 5:6])
        vec.tensor_mul(sc[:, 15:16], sc[:, 15:16], params[:, 2:3])  # A -> col15
        vec.tensor_mul(sc[:, 0:1], sc[:, 12:13], sc[:, 15:16])
        vec.tensor_sub(sc[:, 0:1], params[:, 3:4], sc[:, 0:1])  # B -> col0 (reuse)
        vec.tensor_copy(aba[:, 0:1], sc[:, 15:16])
        vec.tensor_copy(aba[:, 1:2], sc[:, 0:1])
        vec.tensor_copy(aba[:, 2:3], params[:, 4:5])
        for L in range(NL):
            sync.dma_start(out=abx[64 * L : 64 * L + 64, :], in_=aba[:, :])

        # ---------------- final: out = Prelu(yt*A + x + B) ----------------
        for r in range(R):
            ot = stgp.tile([128, HW], F16, tag="ot", name="ot")
            for c, (r0, nr) in enumerate(CHUNKS):
                ut = psp[c].tile([128, nr, 34], F32, tag=f"pt{c}", name=f"ut{c}")
                vec.scalar_tensor_tensor(
                    ut[:, :, 0:32], yt[:, r, 32 * r0 : 32 * (r0 + nr)], abx[:, 0:1],
                    x[:, r, r0 : r0 + nr, :].rearrange("p a b -> p (a b)"),
                    mybir.AluOpType.mult, mybir.AluOpType.add,
                )
                if "prelu" in ablate:  # CoreSim has no Prelu; Copy for sim checks
                    act.activation(
                        ot[:, 32 * r0 : 32 * (r0 + nr)], ut[:, :, 0:32],
                        mybir.ActivationFunctionType.Copy, bias=0.0, scale=1.0,
                    )
                else:
                    act.activation(
                        ot[:, 32 * r0 : 32 * (r0 + nr)], ut[:, :, 0:32],
                        mybir.ActivationFunctionType.Prelu,
                        bias=abx[:, 1:2], scale=1.0, alpha=abx[:, 2:3],
                    )
            sync.dma_start(
                out=bass.AP(out_d, r * HW, [[R * HW, 128], [1, HW]]),
                in_=ot[:, :],
            )

    if loop_n:
        with tc.For_i(0, loop_n, 1):
            emit()
    else:
        emit()

    for p in reversed(pools):
        p.release()


_NC_CACHE = {}


def _get_nc(debug=False):
    if debug not in _NC_CACHE:
        _NC_CACHE[debug] = build_nc(debug)
    return _NC_CACHE[debug]


def make_in_maps(x, conv_w, bn1_gamma, bn1_beta, bn2_gamma, bn2_beta, prelu_a):
    in_maps = []
    for i in range(8):
        cs = slice(64 * i, 64 * (i + 1))
        p = np.stack(
            [bn1_gamma[cs], bn1_beta[cs], bn2_gamma[cs], bn2_beta[cs], prelu_a[cs]]
        ).astype(np.float32)
        xc = np.asarray(x[:, cs])  # [64n, 64c, 32, 32]
        # x1: [(L2,g,ci), r, hw] ; n = 2r+L2, c = 32g+ci
        x1 = np.ascontiguousarray(
            xc.reshape(R, NL, G, 32, HW).transpose(1, 2, 3, 0, 4).reshape(128, R, HW)
        )
        in_maps.append(
            {
                "x1": x1,
                "w": np.ascontiguousarray(conv_w[cs]),
                "p": np.ascontiguousarray(p),
            }
        )
    return in_maps


def gather_out(res_out):
    # res_out: [(L2,g,ci), r, hw] fp16 -> [n, c, h, w] f32
    o = res_out.reshape(NL, G, 32, R, 32, 32).transpose(3, 0, 1, 2, 4, 5)
    return np.ascontiguousarray(o.reshape(N, C, 32, 32).astype(np.float32))


def kernel(x, conv_w, bn1_gamma, bn1_beta, bn2_gamma, bn2_beta, prelu_a, _trace=False):
    from concourse.bass_utils import run_bass_kernel_spmd

    nc = _get_nc()
    in_maps = make_in_maps(
        x, conv_w, bn1_gamma, bn1_beta, bn2_gamma, bn2_beta, prelu_a
    )
    res = run_bass_kernel_spmd(nc, in_maps, list(range(8)), trace=_trace)
    out = np.concatenate(
        [gather_out(res.results[i]["out"]) for i in range(8)], axis=1
    )
    if _trace:
        kernel._last = res
    return out
